# revision 15
# baseline (speedup 1.0000x reference)
"""GNN message-passing kernel for 8 Trainium2 NeuronCores — v3.

Math (per reference):
  h   = relu(ef @ W1 + b1)                      [E, H]
  K   = (h @ W2 + b2).reshape(E, G, L)          per-edge [G, L] kernels
  t   = einsum('bnl,ne->bel', x, inc)           gather nodes->edges
  y   = einsum('egl,bel->beg', K, t)            per-edge matvec
  out = relu(einsum('ne,beg->bng', inc, y) + b_gc).reshape(B, N*G)

Distribution: shard E across 8 cores (2000 edges each, padded to 2048);
scatter partials summed on host.

v3 (all per core, edges in 4 superchunks of 512):
  e_local = sc*512 + p*256 + bh*128 + pair   (p = partition-half bit)
  - mlp2/gather: N=512 matmuls into [128,512] psum; 4 [64,256] drains
    each into kT2/tT2 [(p,l), g|b, pair]; matvec runs both p-halves
    concurrently in PE quadrants (0,0)+(64,64).
  - matvec psum pq [128=(p,g), 8 pair, 64 b] -> ycp [128, b, pair]
    (contiguous 128-col transpose weights => FWL).
  - 128x128 PE transposes -> yfin [pair, (p,b,g)] kept in SBUF.
  - scatter per superchunk (quarter): psum-chains of 4 (2 blk x 2 p)
    read yfin directly (no DRAM staging); lhsT = incPE with n padded
    to 512 (128-col weights => FWL). out[4, 512, B*G] f32; host sums
    quarters, crops n, adds bias + relu.
"""

import numpy as np
import ml_dtypes

import concourse.bass as bass
from concourse import bacc
import concourse.mybir as mybir
import concourse.tile as tile
from concourse.bass_utils import run_bass_kernel_spmd
from concourse.masks import make_identity

B, N, E, L, G, F, H = 64, 500, 16000, 64, 64, 8, 128
NP2 = 512               # n padded for scatter lhsT
NCORES = 8
ELR = E // NCORES       # 2000 real edges per core
EL = 2048               # padded; pad edges have zero incidence
SC = 512                # edges per superchunk
NSC = EL // SC          # 4
PR = 256                # pairs per superchunk
BG = B * G              # 4096
F32 = mybir.dt.float32
BF16 = mybir.dt.bfloat16
RELU = mybir.ActivationFunctionType.Relu
IDENT = mybir.ActivationFunctionType.Identity

_CACHE = {}
last_results = None     # BassKernelResults of the most recent run (for test.py)


def _build():
    nc = bacc.Bacc("TRN2", target_bir_lowering=False)
    xT_d = nc.declare_dram_parameter("xT", [N, B * L], BF16, isOutput=False)
    inc_d = nc.declare_dram_parameter("inc", [N, EL], BF16, isOutput=False)
    incPE_d = nc.declare_dram_parameter("incPE", [128, 8, 2, NP2], BF16, isOutput=False)
    efT_d = nc.declare_dram_parameter("efT", [F, EL], BF16, isOutput=False)
    W1_d = nc.declare_dram_parameter("W1", [F, H], BF16, isOutput=False)
    b1_d = nc.declare_dram_parameter("b1", [H, 1], F32, isOutput=False)
    W2_d = nc.declare_dram_parameter("W2", [H, G * L], BF16, isOutput=False)
    b2T_d = nc.declare_dram_parameter("b2T", [H, G * L // H], F32, isOutput=False)
    out_d = nc.declare_dram_parameter("out", [NSC + 1, NP2, BG], F32, isOutput=True)

    with (
        tile.TileContext(nc) as tc,
        tc.tile_pool(name="const", bufs=1) as cpool,
        tc.tile_pool(name="inct", bufs=2) as ipool,
        tc.tile_pool(name="kt", bufs=1) as ktpool,
        tc.tile_pool(name="tt", bufs=1) as ttpool,
        tc.tile_pool(name="ycp", bufs=1) as ycppool,
        tc.tile_pool(name="yfin", bufs=3) as yfpool,
        tc.tile_pool(name="ot", bufs=2) as otpool,
        tc.tile_pool(name="mm_ps", bufs=3, space="PSUM") as mps,
        tc.tile_pool(name="pqt_ps", bufs=3, space="PSUM") as qps,
        tc.tile_pool(name="sc_ps", bufs=2, space="PSUM") as sps,
    ):
        # ---- small constants first (so mlp1 starts early) ----
        W1_sb = cpool.tile([F, H], BF16)
        nc.sync.dma_start(out=W1_sb[:, :], in_=W1_d[:, :])
        b1_sb = cpool.tile([H, 1], F32)
        nc.sync.dma_start(out=b1_sb[:, :], in_=b1_d[:, :])
        efT_sb = cpool.tile([F, EL], BF16)
        nc.sync.dma_start(out=efT_sb[:, :], in_=efT_d[:, :])
        W2_sb = cpool.tile([H, G * L], BF16)            # 8KB/part
        nc.sync.dma_start(out=W2_sb[:, :], in_=W2_d[:, :])
        b2T_sb = cpool.tile([H, G * L // H], F32)
        nc.sync.dma_start(out=b2T_sb[:, :], in_=b2T_d[:, :])
        hT_sb = cpool.tile([H, EL], BF16)               # 4KB/part
        ident = cpool.tile([128, 128], BF16)
        make_identity(nc, ident[:, :])

        # ---- mlp1: hT = relu(W1.T @ efT + b1) ----
        for c in range(4):
            ph = mps.tile([H, 512], F32, tag="big", name=f"ph{c}")
            nc.tensor.matmul(
                ph[:, :], lhsT=W1_sb[:, :],
                rhs=efT_sb[:, c * 512:(c + 1) * 512],
                start=True, stop=True,
            )
            nc.scalar.activation(
                hT_sb[:, c * 512:(c + 1) * 512], ph[:, :], RELU,
                bias=b1_sb[:, 0:1],
            )

        # ---- big inputs (issued after mlp1 so it isn't blocked) ----
        xT_sb = cpool.tile([125, 4, B * L], BF16)       # 32KB/part
        nc.sync.dma_start(
            out=xT_sb[:, :, :],
            in_=xT_d[:, :].rearrange("(q n) c -> n q c", q=4),
        )
        incPE_sb = cpool.tile([128, 8, 2, NP2], BF16)   # 16KB/part
        nc.sync.dma_start(out=incPE_sb[:, :, :, :], in_=incPE_d[:, :, :, :])

        def scatter_chains(oi, parts):
            # e-partial over `parts` = [(blk, yfin), ...] -> out_d[oi].
            # Generator: yields after each (nj, m) chain so the caller can
            # interleave these drain-light PE chains into the drain-heavy
            # mlp2/gather stream of the next superchunk.
            nmm = 2 * len(parts)
            for nj in range(8):
                for m in range(4):
                    ps = sps.tile([128, 512], F32, tag="ps",
                                  name=f"ps{oi}_{nj}_{m}")
                    k = 0
                    for blk, yf in parts:
                        for p in (0, 1):
                            nc.tensor.matmul(
                                ps[:, :],
                                lhsT=incPE_sb[:, blk, p,
                                              m * 128:(m + 1) * 128],
                                rhs=yf[:, p, nj * 8:(nj + 1) * 8, :],
                                start=(k == 0), stop=(k == nmm - 1),
                            )
                            k += 1
                    ot = otpool.tile([128, 512], F32, tag="ot",
                                     name=f"ot{oi}_{nj}_{m}")
                    if m % 2 == 0:
                        nc.vector.tensor_copy(ot[:, :], ps[:, :])
                    else:
                        nc.scalar.copy(ot[:, :], ps[:, :])
                    nc.sync.dma_start(
                        out=out_d[oi, m * 128:(m + 1) * 128,
                                  nj * 512:(nj + 1) * 512],
                        in_=ot[:, :],
                    )
                    yield

        prev_scatter = None     # generator of previous quarter's chains

        def drive(gen):
            if gen is not None:
                next(gen, None)

        for sc in range(NSC):
            e0 = sc * SC
            # ---- mlp2 -> kT2[(p,l), g, pair] (+ b2 bias) ----
            kT2 = ktpool.tile([128, G, PR], BF16, tag="kt", name=f"kt{sc}")
            for mc in range(32):
                pm = mps.tile([H, 512], F32, tag="big", name=f"pm{sc}_{mc}")
                nc.tensor.matmul(
                    pm[:, :], lhsT=W2_sb[:, mc * H:(mc + 1) * H],
                    rhs=hT_sb[:, e0:e0 + SC], start=True, stop=True,
                )
                for par in (0, 1):
                    bias = b2T_sb[par * 64:(par + 1) * 64, mc:mc + 1]
                    for p in (0, 1):
                        src = pm[par * 64:(par + 1) * 64, p * 256:(p + 1) * 256]
                        dst = kT2[p * 64:(p + 1) * 64, 2 * mc + par, :]
                        if p == 0:
                            nc.scalar.activation(dst, src, IDENT, bias=bias)
                        else:
                            nc.vector.tensor_scalar_add(dst, src, bias)
                if mc % 2 == 0:
                    drive(prev_scatter)

            # ---- gather -> tT2[(p,l), b, pair] ----
            inc_t = ipool.tile([125, 4, SC], BF16, tag="inc", name=f"inc{sc}")
            nc.sync.dma_start(
                out=inc_t[:, :, :],
                in_=inc_d[:, e0:e0 + SC].rearrange("(q n) e -> n q e", q=4),
            )
            tT2 = ttpool.tile([128, B, PR], BF16, tag="tt", name=f"tt{sc}")
            for bp in range(32):
                pg = mps.tile([128, 512], F32, tag="big", name=f"pg{sc}_{bp}")
                for q in range(4):
                    nc.tensor.matmul(
                        pg[:, :],
                        lhsT=xT_sb[:, q, bp * 128:(bp + 1) * 128],
                        rhs=inc_t[:, q, :],
                        start=(q == 0), stop=(q == 3),
                    )
                for b01 in (0, 1):
                    for p in (0, 1):
                        src = pg[b01 * 64:(b01 + 1) * 64, p * 256:(p + 1) * 256]
                        dst = tT2[p * 64:(p + 1) * 64, 2 * bp + b01, :]
                        if p == 0:
                            nc.scalar.copy(dst, src)
                        else:
                            nc.vector.tensor_copy(dst, src)
                if bp % 2 == 0:
                    drive(prev_scatter)

            # finish any chains of the previous quarter not yet emitted
            if prev_scatter is not None:
                for _ in prev_scatter:
                    pass
                prev_scatter = None

            # ---- matvec (p-halves concurrent) + transposes -> yfin ----
            last = sc == NSC - 1
            blk6 = blk7 = None
            yfins = []
            for bh in range(2):
                ycp = ycppool.tile([128, 128, B], BF16, tag="ycp",
                                   name=f"ycp{sc}_{bh}")
                for pr8 in range(16):
                    pq = qps.tile([128, 8, B], F32, tag="pqt",
                                  name=f"pq{sc}_{bh}_{pr8}")
                    for k in range(8):
                        pr = bh * 128 + pr8 * 8 + k
                        for p in (0, 1):
                            nc.tensor.matmul(
                                pq[p * 64:(p + 1) * 64, k, :],
                                lhsT=kT2[p * 64:(p + 1) * 64, :, pr],
                                rhs=tT2[p * 64:(p + 1) * 64, :, pr],
                                start=True, stop=True,
                            )
                    dst = ycp[:, pr8 * 8:(pr8 + 1) * 8, :]
                    if pr8 % 2 == 0:
                        nc.scalar.copy(dst, pq[:, :, :])
                    else:
                        nc.vector.tensor_copy(dst, pq[:, :, :])
                    if bh == 1 and blk6 is not None:
                        drive(blk6)
                        drive(blk6)

                yfin = yfpool.tile([128, 2, B, G], BF16, tag="yf",
                                   name=f"yf{sc}_{bh}")
                if last and bh == 1:
                    # chain (nj, m) only reads yfin b-block nj*8:(nj+1)*8,
                    # produced by the b8 == nj transpose drain below.
                    blk7 = scatter_chains(NSC, [(2 * sc + 1, yfin)])
                for b8 in range(8):
                    pt = qps.tile([128, 2, 8, G], BF16, tag="pqt",
                                  name=f"pt{sc}_{bh}_{b8}")
                    for i in range(8):
                        b = b8 * 8 + i
                        nc.tensor.transpose(
                            pt[:, :, i, :], ycp[:, :, b], ident[:, :],
                        )
                    dst = yfin[:, :, b8 * 8:(b8 + 1) * 8, :]
                    if b8 % 2 == 0:
                        nc.vector.tensor_copy(dst, pt[:, :, :, :])
                    else:
                        nc.scalar.copy(dst, pt[:, :, :, :])
                    if last and bh == 1:
                        for _ in range(4):
                            drive(blk7)
                    elif bh == 1 and blk6 is not None:
                        drive(blk6)
                yfins.append(yfin)
                if last and bh == 0:
                    blk6 = scatter_chains(NSC - 1, [(2 * sc, yfin)])

            if last:
                for _ in blk6:
                    pass
                for _ in blk7:
                    pass
            else:
                prev_scatter = scatter_chains(sc, [(2 * sc, yfins[0]),
                                                   (2 * sc + 1, yfins[1])])

    nc.compile()
    return nc


def kernel(x, incidence, ef, W1, b1, W2, b2, b_gc):
    global last_results
    x = np.asarray(x, dtype=np.float32)
    incidence = np.asarray(incidence, dtype=np.float32)
    ef = np.asarray(ef, dtype=np.float32)
    W1 = np.asarray(W1, dtype=np.float32)
    b1 = np.asarray(b1, dtype=np.float32)
    W2 = np.asarray(W2, dtype=np.float32)
    b2 = np.asarray(b2, dtype=np.float32)
    b_gc = np.asarray(b_gc, dtype=np.float32)

    if "nc" not in _CACHE:
        _CACHE["nc"] = _build()
    nc = _CACHE["nc"]

    bf = ml_dtypes.bfloat16
    xT = np.ascontiguousarray(
        x.transpose(1, 0, 2).reshape(N, B * L)).astype(bf)
    inc_bf = incidence.astype(bf)
    b1c = np.ascontiguousarray(b1.reshape(H, 1))
    W2_bf = W2.astype(bf)
    b2T = np.ascontiguousarray(b2.reshape(G * L // H, H).T)
    efT_full = np.ascontiguousarray(ef.T).astype(bf)

    pad = EL - ELR
    in_maps = []
    for c in range(NCORES):
        es = slice(c * ELR, (c + 1) * ELR)
        inc_c = np.pad(inc_bf[:, es], ((0, 0), (0, pad)))        # [N, EL]
        # incPE[pair, blk, p, n_pad];  blk = sc*2 + bh
        inc_cp = np.pad(inc_c, ((0, NP2 - N), (0, 0)))           # [512, EL]
        e_idx = (np.arange(4)[:, None, None] * 512
                 + np.arange(2)[None, :, None] * 256
                 + np.arange(256)[None, None, :])
        e_idx = e_idx.reshape(4, 2, 2, 128)                      # [sc, p, bh, pair]
        incPE = inc_cp.T[e_idx]                                  # [sc, p, bh, pair, 512]
        incPE = np.ascontiguousarray(
            incPE.transpose(3, 0, 2, 1, 4).reshape(128, 8, 2, NP2))
        in_maps.append({
            "xT": xT,
            "inc": np.ascontiguousarray(inc_c),
            "incPE": incPE,
            "efT": np.ascontiguousarray(
                np.pad(efT_full[:, es], ((0, 0), (0, pad)))),
            "W1": W1.astype(bf), "b1": b1c, "W2": W2_bf, "b2T": b2T,
        })

    import os
    trace = bool(int(os.environ.get("KERNEL_TRACE", "0")))
    last_results = run_bass_kernel_spmd(
        nc, in_maps, list(range(NCORES)), trace=trace)
    partial = np.zeros((N, B, G), np.float32)
    for r in last_results.results:
        o = r["out"]                                        # [4, 512, BG]
        partial += o.sum(axis=0)[:N].reshape(N, B, G)
    out = np.maximum(partial.transpose(1, 0, 2)
                     + b_gc.reshape(1, 1, G), 0.0)
    return out.reshape(B, N * G).astype(np.float32)
